# revision 1
# baseline (speedup 1.0000x reference)
"""AttentionBlock3D on 8 Trainium2 NeuronCores.

Math (see reference.py): GroupNorm(8 groups) -> qkv 1x1 conv -> channel
attention (contract over tokens N, softmax over last d=64) -> out proj ->
residual.

Sharding: N = T*H*W = 16384 tokens split 8 ways (2048/core); every core
holds all 4 batches of its token slice.  Cross-core state is tiny and
handled with two AllReduces: GroupNorm per-channel sum/sumsq (16 KB) and
the attention logits (512 KB = all (b,head) 64x64 blocks).

Per-core phases:
  A: stream x per (b, co-block) [128 x n_loc], bn_stats -> per-channel
     sum/sumsq -> AllReduce -> group stats via a PE partition-reduce ->
     GN folded to h = a*x + c (per-channel scale/shift).
     The Q/K biases are folded into rank-1 logit corrections built from
     the global channel sums (avoids 128 expensive K=1 fp32 matmuls).
  B: per 512-token chunk: h (fp32r, ACT), QK^T projection with tokens on
     PSUM partitions (so logits need no transpose), logits accumulated
     into per-head-pair PSUM banks (heads paired via col-tiling)
     -> logits AllReduce
  C: V projection (first PIPE_LAG chunks emitted *before* softmax so the
     AllReduce hides behind them), softmax + PE blockdiag-transpose of
     attn, then attn@v + out projection + bias + residual, store.

Projections run in float32r (TF32-like, 4x faster PE); the logit matmuls
(error-amplified by softmax) run in exact fp32.
"""

import numpy as np

import concourse.bass as bass
import concourse.mybir as mybir
import concourse.tile as tile
from concourse import bass_utils

F32 = mybir.dt.float32
F32R = mybir.dt.float32r
AX = mybir.AxisListType.X
ALU = mybir.AluOpType
ACT = mybir.ActivationFunctionType

N_CORES = 8
B, C, T, H, W = 4, 512, 16, 32, 32
N_TOT = T * H * W            # 16384
NH, D = 8, 64                # heads, head dim
G = 8                        # groupnorm groups
EPS = 1e-5
P = 128
CO = C // P                  # 4 channel chunks
NC = 512                     # token chunk size
PIPE_LAG = 3                 # attn@v lags V-proj by this many chunks


def _round_tf32(a: np.ndarray) -> np.ndarray:
    """Round fp32 to fp32r (keep 10 explicit mantissa bits, RNE)."""
    u = a.astype(np.float32).view(np.uint32).astype(np.uint64)
    u = (u + 0x1000 + ((u >> 13) & 1)) & 0xFFFFE000
    return u.astype(np.uint32).view(np.float32)


def build_module(n_loc: int, debug: bool = False):
    nchunks = n_loc // NC
    ngr = n_loc // 512           # bn_stats groups per (b, co) row
    ntot = n_loc * N_CORES
    m_group = (C // G) * ntot    # elements per (b, group) stat
    scale = float(D) ** -0.5

    nc = bass.Bass("TRN2", target_bir_lowering=False, debug=False,
                   num_devices=N_CORES)

    xin = nc.dram_tensor("xin", [B, C, n_loc], F32, kind="ExternalInput").ap()
    wqk_t = nc.dram_tensor("wqk_t", [C, 2 * C], F32, kind="ExternalInput").ap()
    wv_t = nc.dram_tensor("wv_t", [C, C], F32, kind="ExternalInput").ap()
    wo_t = nc.dram_tensor("wo_t", [C, C], F32, kind="ExternalInput").ap()
    qkb = nc.dram_tensor("qkb", [1, 2 * C], F32, kind="ExternalInput").ap()
    vb2 = nc.dram_tensor("vb2", [P, CO], F32, kind="ExternalInput").ap()
    ob2 = nc.dram_tensor("ob2", [P, CO], F32, kind="ExternalInput").ap()
    gnw2 = nc.dram_tensor("gnw2", [P, CO], F32, kind="ExternalInput").ap()
    gnb2 = nc.dram_tensor("gnb2", [P, CO], F32, kind="ExternalInput").ap()
    yout = nc.dram_tensor("yout", [B, C, n_loc], F32, kind="ExternalOutput").ap()
    dbg = {}
    if debug:
        for nm, shp in [("dbg_stats", [P, 2, CO, B]), ("dbg_a", [P, CO, B]),
                        ("dbg_c", [P, CO, B]), ("dbg_qk", [P, 2 * C]),
                        ("dbg_logits", [P, B, 4, D]), ("dbg_attn", [P, B, 4, D]),
                        ("dbg_sq", [B, 2 * C]), ("dbg_v", [P, CO, NC]),
                        ("dbg_av", [P, CO, NC])]:
            dbg[nm] = nc.dram_tensor(nm, shp, F32, kind="ExternalOutput").ap()

    import contextlib
    with tile.TileContext(nc) as tc:
        with (
            tc.tile_pool(name="persist", bufs=1) as pers,
            tc.tile_pool(name="dram", bufs=1, space="DRAM") as dram,
            tc.tile_pool(name="xp", bufs=PIPE_LAG + 3) as xp,
        ):
            prep_stack = contextlib.ExitStack()
            small = prep_stack.enter_context(
                tc.tile_pool(name="prep", bufs=2))
            # ---------------- persistent tiles -----------------
            wqk_sb = pers.tile([P, CO, 2 * C], F32R)
            wv_sb = pers.tile([P, CO, C], F32R)
            wo_sb = pers.tile([P, CO, C], F32R)
            nc.gpsimd.dma_start(
                wqk_sb[:], wqk_t.rearrange("(co ci) o -> ci co o", ci=P))
            nc.gpsimd.dma_start(
                wv_sb[:], wv_t.rearrange("(co ci) o -> ci co o", ci=P))
            nc.gpsimd.dma_start(
                wo_sb[:], wo_t.rearrange("(co ci) o -> ci co o", ci=P))

            qkb_sb = pers.tile([1, 2 * C], F32)
            nc.sync.dma_start(qkb_sb[:], qkb[:])
            vb_sb = pers.tile([P, CO], F32)
            ob_sb = pers.tile([P, CO], F32)
            gnw_sb = pers.tile([P, CO], F32)
            gnb_sb = pers.tile([P, CO], F32)
            nc.sync.dma_start(vb_sb[:], vb2[:])
            nc.sync.dma_start(ob_sb[:], ob2[:])
            nc.sync.dma_start(gnw_sb[:], gnw2[:])
            nc.sync.dma_start(gnb_sb[:], gnb2[:])

            ident = pers.tile([P, P], F32)
            from concourse.masks import make_identity
            make_identity(nc, ident[:])
            sel_sb = pers.tile([P, 2], F32)
            nc.vector.memset(sel_sb[:], 0.0)
            nc.vector.memset(sel_sb[0:64, 0:1], 1.0)
            nc.vector.memset(sel_sb[64:128, 1:2], 1.0)
            selt_sb = pers.tile([2, P], F32)
            with tc.tile_pool(name="selps", bufs=1, space="PSUM") as selps:
                sel_pt = selps.tile([2, P], F32)
                nc.tensor.transpose(sel_pt[:], sel_sb[:], ident[:])
                nc.vector.tensor_copy(selt_sb[:], sel_pt[:])

            a_sb = pers.tile([P, CO, B], F32)     # GN scale per (ci,co,b)
            c_sb = pers.tile([P, CO, B], F32)     # GN shift
            logits_sb = pers.tile([P, B, 4, D], F32)
            attn_sb = pers.tile([P, B, 4, D], F32)
            abd_f = pers.tile([P, B * 4, P], F32)  # block-diag attn (untransp.)
            abd_r = pers.tile([P, B * 4, P], F32R)  # transposed, f32r
            ssq = pers.tile([1, B, C], F32)       # Sq_global/8 per (b, o)
            svk = pers.tile([1, B, C], F32)       # (Sk_global + N*bk)/8

            # ------- pipelined per-batch: stats(b) -> AllReduce(b) -> prep(b)
            # b=0 runs up front (DVE/ACT); b>=1 stats run on the otherwise
            # idle GPSIMD engine + ACT-DGE loads so they hide under B(b-1).
            bst = pers.tile([P, B, CO, ngr, 6], F32)
            stats = pers.tile([P, 2, CO, B], F32)  # [ci, stat, co, b] local
            stg = pers.tile([P, 2, CO, B], F32)    # global per-channel sums
            st_in_l = [dram.tile([P, 2, CO], F32, name=f"st_in{bb}")
                       for bb in range(B)]
            st_out_l = [dram.tile([P, 2, CO], F32, name=f"st_out{bb}")
                        for bb in range(B)]
            sqp = prep_stack.enter_context(tc.tile_pool(name="sqp", bufs=2))
            from bass_rust import add_dep_helper as _adh

            def a_stats(b):
                for co in range(CO):
                    xa = sqp.tile([P, n_loc], F32, tag="xA", name="xa")
                    if b == 0:
                        eng = nc.sync if co % 2 == 0 else nc.scalar
                        eng.dma_start(
                            xa[:], xin[b, co * P:(co + 1) * P, :])
                        if co % 2 == 1:
                            nc.scalar.activation(
                                xa[:], xa[:], ACT.Identity,
                                accum_out=stats[:, 0, co, b:b + 1])
                            nc.scalar.activation(
                                xa[:], xa[:], ACT.Square,
                                accum_out=stats[:, 1, co, b:b + 1])
                        else:
                            for g in range(ngr):
                                nc.vector.bn_stats(
                                    bst[:, b, co, g],
                                    xa[:, g * 512:(g + 1) * 512])
                            mvt = small.tile([P, 2], F32, tag="mvt")
                            nc.vector.bn_aggr(mvt[:], bst[:, b, co])
                            nc.vector.tensor_scalar_mul(
                                stats[:, 0, co, b:b + 1], mvt[:, 0:1],
                                float(n_loc))
                            nc.vector.tensor_tensor(
                                stats[:, 1, co, b:b + 1], mvt[:, 0:1],
                                mvt[:, 0:1], ALU.mult)
                            nc.vector.tensor_tensor(
                                stats[:, 1, co, b:b + 1],
                                stats[:, 1, co, b:b + 1], mvt[:, 1:2],
                                ALU.add)
                            nc.vector.tensor_scalar_mul(
                                stats[:, 1, co, b:b + 1],
                                stats[:, 1, co, b:b + 1], float(n_loc))
                    else:
                        # ACT is nearly idle during B eras (h runs on DVE):
                        # sums via accumulating Identity/Square passes
                        nc.scalar.dma_start(
                            xa[:], xin[b, co * P:(co + 1) * P, :])
                        nc.scalar.activation(
                            xa[:], xa[:], ACT.Identity,
                            accum_out=stats[:, 0, co, b:b + 1])
                        nc.scalar.activation(
                            xa[:], xa[:], ACT.Square,
                            accum_out=stats[:, 1, co, b:b + 1])

            def arst(b):
                nc.gpsimd.dma_start(st_in_l[b][:], stats[:, :, :, b])
                nc.gpsimd.collective_compute(
                    "AllReduce", ALU.add,
                    replica_groups=[list(range(N_CORES))],
                    ins=[st_in_l[b].opt()], outs=[st_out_l[b].opt()],
                )
                nc.gpsimd.dma_start(stg[:, :, :, b], st_out_l[b][:])

            def prep(b, qs_pool, pins=None):
                pe_pin, dve_pin, act_pin = pins or (None, None, None)

                def pin(inst, ref):
                    if ref is not None:
                        _adh(inst.ins, ref.ins,
                             reason="pipeline prep stays after prev batch")

                pt1 = qs_pool.tile([P, 2 * C], F32, tag="qk", name="prep_ps")
                mm1 = nc.tensor.matmul(
                    pt1[0:2, 0:2 * CO], sel_sb[:],
                    stg[:, :, :, b].rearrange("p a b -> p (a b)"),
                    start=True, stop=True, skip_group_check=True)
                pin(mm1, pe_pin)
                gst = small.tile([2, 2, CO], F32, tag="gst")
                cp1 = nc.vector.tensor_copy(
                    gst[:].rearrange("p a b -> p (a b)"), pt1[0:2, 0:2 * CO])
                pin(cp1, dve_pin)
                mean_t = small.tile([2, CO], F32, tag="mean")
                nc.vector.tensor_scalar_mul(mean_t[:], gst[:, 0],
                                            1.0 / m_group)
                ex2_t = small.tile([2, CO], F32, tag="ex2")
                nc.vector.tensor_scalar_mul(ex2_t[:], gst[:, 1],
                                            1.0 / m_group)
                var_t = small.tile([2, CO], F32, tag="var")
                nc.vector.tensor_tensor(var_t[:], mean_t[:], mean_t[:],
                                        ALU.mult)
                nc.vector.tensor_tensor(var_t[:], ex2_t[:], var_t[:],
                                        ALU.subtract)
                rstd_t = small.tile([2, CO], F32, tag="rstd")
                sq1 = nc.scalar.activation(rstd_t[:], var_t[:], ACT.Sqrt,
                                           bias=eps_t[:])
                pin(sq1, act_pin)
                nc.vector.reciprocal(rstd_t[:], rstd_t[:])
                cg_t = small.tile([2, CO], F32, tag="cg")
                nc.vector.tensor_tensor(cg_t[:], mean_t[:], rstd_t[:],
                                        ALU.mult)
                nc.vector.tensor_scalar_mul(cg_t[:], cg_t[:], -1.0)
                rc2 = small.tile([2, 2, CO], F32, tag="rc2")
                nc.vector.tensor_copy(rc2[:, 0], rstd_t[:])
                nc.vector.tensor_copy(rc2[:, 1], cg_t[:])
                nc.tensor.matmul(
                    pt1[:, 512:512 + 2 * CO], selt_sb[:],
                    rc2[:].rearrange("p a b -> p (a b)"),
                    start=True, stop=True, skip_group_check=True)
                bc = small.tile([P, 2, CO], F32, tag="bc")
                nc.vector.tensor_copy(
                    bc[:].rearrange("p a b -> p (a b)"),
                    pt1[:, 512:512 + 2 * CO])
                nc.vector.tensor_tensor(a_sb[:, :, b], bc[:, 0], gnw_sb[:],
                                        ALU.mult)
                nc.vector.tensor_tensor(c_sb[:, :, b], bc[:, 1], gnw_sb[:],
                                        ALU.mult)
                nc.vector.tensor_tensor(c_sb[:, :, b], c_sb[:, :, b],
                                        gnb_sb[:], ALU.add)
                # Sh = a*Sx + c*ntot (f32r), then Sq/Sk = Wqk^T @ Sh
                sh_r = small.tile([P, CO], F32R, tag="shr")
                nc.vector.tensor_tensor(sh_r[:], a_sb[:, :, b],
                                        stg[:, 0, :, b], ALU.mult)
                tmp_c = small.tile([P, CO], F32R, tag="tmpc")
                nc.vector.tensor_scalar_mul(tmp_c[:], c_sb[:, :, b],
                                            float(ntot))
                nc.vector.tensor_tensor(sh_r[:], sh_r[:], tmp_c[:], ALU.add)
                pt2 = qs_pool.tile([P, 2 * C], F32, tag="qk", name="prep_ps2")
                for half in range(2):
                    sl = slice(half * 512, half * 512 + 512)
                    for co in range(CO):
                        nc.tensor.matmul(
                            pt2[0:1, sl], sh_r[:, co:co + 1],
                            wqk_sb[:, co, sl],
                            start=(co == 0), stop=(co == CO - 1),
                            skip_group_check=True)
                sq1t = small.tile([1, 2 * C], F32, tag="sq1")
                nc.vector.tensor_copy(sq1t[:], pt2[0:1, :])
                nc.vector.tensor_scalar_mul(
                    ssq[:, b], sq1t[:, 0:C], 1.0 / N_CORES)
                nc.vector.tensor_scalar_mul(
                    svk[:, b], qkb_sb[:, C:2 * C], float(ntot) / N_CORES)
                tmp_s = small.tile([1, C], F32, tag="tmps")
                nc.vector.tensor_scalar_mul(
                    tmp_s[:], sq1t[:, C:2 * C], 1.0 / N_CORES)
                nc.vector.tensor_tensor(svk[:, b], svk[:, b], tmp_s[:],
                                        ALU.add)

            eps_t = small.tile([2, 1], F32)
            nc.vector.memset(eps_t[:], EPS)
            if debug:
                pass  # dbg_stats/a/c handled after loop

            # ---------------- phase B: QK^T + logits -----------------
            def make_h(hpool, xa, b, tag):
                h = hpool.tile([P, CO, NC], F32R, tag=tag)
                last = None
                for co in range(CO):
                    last = nc.vector.tensor_scalar(
                        h[:, co], xa[:, co],
                        a_sb[:, co, b:b + 1], c_sb[:, co, b:b + 1],
                        ALU.mult, ALU.add)
                return h, last

            lg_full = pers.tile([P, B, 4, D], F32)
            lg_in_l = [dram.tile([P, 4, D], F32, name=f"lg_in{bb}")
                       for bb in range(B)]
            lg_out_l = [dram.tile([P, 4, D], F32, name=f"lg_out{bb}")
                        for bb in range(B)]
            with (
                tc.tile_pool(name="hpb", bufs=4) as hpb,
                tc.tile_pool(name="qkp", bufs=3) as qkp,
                tc.tile_pool(name="qs", bufs=2, space="PSUM") as qs,
                tc.tile_pool(name="lgps", bufs=1, space="PSUM") as lgps,
            ):
                a_stats(0)
                arst(0)
                prep(0, qs)
                for b in range(B):
                    if b + 1 < B:
                        a_stats(b + 1)
                        arst(b + 1)
                    xv = xin[b].rearrange("(co ci) n -> ci co n", ci=P)
                    lg_ps = [lgps.tile([P, D], F32, tag=f"lg{hh}",
                                       name=f"lg{hh}") for hh in range(4)]

                    def corrections(b=b, lg_ps=lg_ps):
                        # rank-1 bias corrections (start=True clears regions):
                        #   logits += bq[d]*svk[e] + ssq[d]*bk[e]
                        for hp_i in range(4):
                            for par in range(2):
                                hoff = hp_i * P + par * D
                                rows = slice(par * D, par * D + D)
                                tp = (0, 64) if par else None
                                nc.tensor.matmul(
                                    lg_ps[hp_i][rows, :],
                                    qkb_sb[:, hoff:hoff + D],
                                    svk[:, b, hoff:hoff + D],
                                    start=True, stop=False, tile_position=tp,
                                    skip_group_check=True)
                                nc.tensor.matmul(
                                    lg_ps[hp_i][rows, :],
                                    ssq[:, b, hoff:hoff + D],
                                    qkb_sb[:, C + hoff:C + hoff + D],
                                    start=False, stop=False, tile_position=tp,
                                    skip_group_check=True)
                    pending = None
                    micro = 0
                    for j in range(nchunks):
                        xa = xp.tile([P, CO, NC], F32, tag="x")
                        nc.sync.dma_start(xa[:], xv[:, :, j * NC:(j + 1) * NC])
                        h, _ = make_h(hpb, xa, b, "hb")
                        for t in range(NC // P):
                            ps_qk = qs.tile([P, 2 * C], F32, tag="qk")
                            for half in range(2):
                                sl = slice(half * 512, half * 512 + 512)
                                for co in range(CO):
                                    nc.tensor.matmul(
                                        ps_qk[:, sl],
                                        h[:, co, t * P:(t + 1) * P],
                                        wqk_sb[:, co, sl],
                                        start=(co == 0),
                                        stop=(co == CO - 1),
                                    )
                            qk_sb = qkp.tile([P, 2 * C], F32, tag="qks")
                            last_qk_copy = nc.vector.tensor_copy(
                                qk_sb[:, 0:1024], ps_qk[:])
                            if debug and b == 0 and micro == 0:
                                nc.sync.dma_start(dbg["dbg_qk"][:], qk_sb[:])
                            if micro == 0:
                                corrections()
                            if pending is not None:
                                _logits_mms(nc, lg_ps, pending)
                            pending = qk_sb
                            micro += 1
                    last_lg_mm = _logits_mms(nc, lg_ps, pending, last=True)
                    last_evac = None
                    for hh in range(4):
                        last_evac = nc.scalar.copy(logits_sb[:, b, hh],
                                                   lg_ps[hh][:])
                    # per-batch logits AllReduce: b<3 reductions overlap the
                    # remaining batches' compute; only b=3's is exposed
                    nc.gpsimd.dma_start(lg_in_l[b][:], logits_sb[:, b])
                    nc.gpsimd.collective_compute(
                        "AllReduce", ALU.add,
                        replica_groups=[list(range(N_CORES))],
                        ins=[lg_in_l[b].opt()], outs=[lg_out_l[b].opt()],
                    )
                    nc.gpsimd.dma_start(lg_full[:, b], lg_out_l[b][:])
                    if b + 1 < B:
                        prep(b + 1, qs,
                             pins=(last_lg_mm, last_qk_copy, last_evac))
            prep_stack.close()
            if debug:
                nc.sync.dma_start(dbg["dbg_a"][:], a_sb[:])
                nc.sync.dma_start(dbg["dbg_c"][:], c_sb[:])
                nc.sync.dma_start(dbg["dbg_stats"][:], stg[:])
            if debug:
                nc.sync.dma_start(dbg["dbg_logits"][:], lg_full[:])

            # ---------------- phase C (+ softmax after LAG V-chunks) -------
            chunks = [(b, j) for b in range(B) for j in range(nchunks)]
            v_tiles, x_tiles = {}, {}

            with (
                tc.tile_pool(name="hpc", bufs=2) as hpc,
                tc.tile_pool(name="vp", bufs=PIPE_LAG + 2) as vp,
                tc.tile_pool(name="avp", bufs=1) as avp,
                tc.tile_pool(name="yp", bufs=4) as yp,
                tc.tile_pool(name="cps", bufs=5, space="PSUM") as cps,
            ):
                def emit_v(idx):
                    b, j = chunks[idx]
                    xv = xin[b].rearrange("(co ci) n -> ci co n", ci=P)
                    xa = xp.tile([P, CO, NC], F32, tag="x")
                    nc.sync.dma_start(xa[:], xv[:, :, j * NC:(j + 1) * NC])
                    x_tiles[idx] = xa
                    h, h_last = make_h(hpc, xa, b, "hc")
                    emit_v.last_h = h_last
                    # fold the out-proj bias into the residual tile (in-place,
                    # ACT; ordered after make_h's reads of raw x by Tile)
                    for ot in range(CO):
                        nc.scalar.activation(
                            xa[:, ot], xa[:, ot], ACT.Identity,
                            bias=ob_sb[:, ot:ot + 1])
                    v = vp.tile([P, CO, NC], F32R, tag="v")
                    first_mm = last_mm = None
                    for ot in range(CO):
                        ps_v = cps.tile([P, NC], F32, tag="c")
                        for co in range(CO):
                            last_mm = nc.tensor.matmul(
                                ps_v[:], wv_sb[:, co, ot * P:(ot + 1) * P],
                                h[:, co], start=(co == 0), stop=(co == CO - 1))
                            if first_mm is None:
                                first_mm = last_mm
                        nc.vector.tensor_scalar_add(
                            v[:, ot], ps_v[:], vb_sb[:, ot:ot + 1])
                    v_tiles[idx] = v
                    if debug and idx == 0:
                        nc.gpsimd.dma_start(dbg["dbg_v"][:], v[:])
                    return first_mm, last_mm

                def finish(k):
                    bk, jk = chunks[k]
                    vk = v_tiles.pop(k)
                    xk = x_tiles.pop(k)
                    av = avp.tile([P, CO, NC], F32R, tag="av")
                    for ot in range(CO):
                        ps_a = cps.tile([P, NC], F32, tag="c")
                        nc.tensor.matmul(ps_a[:], abd_r[:, bk * 4 + ot],
                                         vk[:, ot], start=True, stop=True)
                        nc.scalar.copy(av[:, ot], ps_a[:])
                    if debug and k == 0:
                        nc.gpsimd.dma_start(dbg["dbg_av"][:], av[:])
                    yv = yout[bk].rearrange("(co ci) n -> ci co n", ci=P)
                    for ot in range(CO):
                        ps_o = cps.tile([P, NC], F32, tag="c")
                        for co in range(CO):
                            nc.tensor.matmul(
                                ps_o[:], wo_sb[:, co, ot * P:(ot + 1) * P],
                                av[:, co], start=(co == 0),
                                stop=(co == CO - 1))
                        y_sb = yp.tile([P, NC], F32, tag="y")
                        nc.vector.tensor_tensor(
                            y_sb[:], ps_o[:], xk[:, ot], ALU.add)
                        nc.sync.dma_start(
                            yv[:, ot, jk * NC:(jk + 1) * NC], y_sb[:])

                # V for the first PRE chunks: overlaps the logits AllReduce
                PRE = PIPE_LAG + 2
                lag_mms = []
                for idx in range(min(PRE, len(chunks))):
                    lag_mms.append(emit_v(idx)[1])

                # softmax + blockdiag + PE transpose
                nc.vector.memset(abd_f[:], 0.0)
                from bass_rust import add_dep_helper
                with tc.tile_pool(name="smp", bufs=4) as smp:
                    for b in range(B):
                        for hp_i in range(4):
                            blk = lg_full[:, b, hp_i]
                            mx = smp.tile([P, 1], F32, tag="mx")
                            nc.vector.reduce_max(mx[:], blk, AX)
                            nbias = smp.tile([P, 1], F32, tag="nb")
                            nc.vector.tensor_scalar_mul(nbias[:], mx[:],
                                                        -scale)
                            ex = attn_sb[:, b, hp_i]
                            exi = nc.scalar.activation(ex, blk, ACT.Exp,
                                                       bias=nbias[:],
                                                       scale=scale)
                            if b == 0 and hp_i == 0 and \
                                    getattr(emit_v, "last_h", None) is not None:
                                add_dep_helper(
                                    exi.ins, emit_v.last_h.ins,
                                    reason="softmax ACT ops after the "
                                           "AllReduce-hiding V chunks")
                            sm = smp.tile([P, 1], F32, tag="sm")
                            nc.vector.reduce_sum(sm[:], ex, AX)
                            nc.vector.reciprocal(sm[:], sm[:])
                            nc.vector.tensor_scalar_mul(ex, ex, sm[:])
                            idx = b * 4 + hp_i
                            nc.vector.tensor_copy(abd_f[0:64, idx, 0:64],
                                                  attn_sb[0:64, b, hp_i])
                            nc.vector.tensor_copy(abd_f[64:128, idx, 64:128],
                                                  attn_sb[64:128, b, hp_i])
                last_tr = None
                with tc.tile_pool(name="tps", bufs=2, space="PSUM") as tps:
                    for idx in range(B * 4):
                        pt = tps.tile([P, P], F32, tag="pt")
                        tr = nc.tensor.transpose(pt[:], abd_f[:, idx, :],
                                                 ident[:])
                        last_tr = tr
                        if idx == 0:
                            for mm in lag_mms:
                                add_dep_helper(
                                    tr.ins, mm.ins,
                                    reason="keep PE transposes after the "
                                           "AllReduce-hiding V chunks")
                        nc.scalar.copy(abd_r[:, idx], pt[:])
                if debug:
                    nc.sync.dma_start(dbg["dbg_attn"][:], attn_sb[:])

                next_fin = [0]

                def finish_up_to(k):
                    while next_fin[0] <= k:
                        finish(next_fin[0])
                        next_fin[0] += 1

                finish_up_to(min(PRE, len(chunks)) - PIPE_LAG - 1)
                for idx in range(PRE, len(chunks)):
                    first_mm, _ = emit_v(idx)
                    if idx == PRE and last_tr is not None:
                        add_dep_helper(
                            first_mm.ins, last_tr.ins,
                            reason="post-softmax V chunks stay after the "
                                   "attn transposes")
                    finish_up_to(idx - PIPE_LAG)
                finish_up_to(len(chunks) - 1)

    return nc


def _logits_mms(nc, lg_ps, qk_sb, last=False):
    mm = None
    for hp_i in range(4):
        q_e = qk_sb[:, hp_i * P: hp_i * P + 64]
        q_o = qk_sb[:, hp_i * P + 64: hp_i * P + 128]
        k_e = qk_sb[:, 512 + hp_i * P: 512 + hp_i * P + 64]
        k_o = qk_sb[:, 512 + hp_i * P + 64: 512 + hp_i * P + 128]
        nc.tensor.matmul(lg_ps[hp_i][0:64, :], q_e, k_e,
                         start=False, stop=last, skip_group_check=True)
        mm = nc.tensor.matmul(lg_ps[hp_i][64:128, :], q_o, k_o,
                              start=False, stop=last,
                              tile_position=(0, 64), skip_group_check=True)
    return mm



_WAITSPLIT_COUNTER = [0]


def _split_waits(nc, limit: int = 1):
    """Walrus in this container rejects instructions with more than one sync
    wait; split extras onto injected NoOps on the same engine."""
    n_split = 0
    for fn in nc.m.functions:
        for bb in fn.blocks:
            insts = list(bb.instructions)
            out = []
            changed = False
            for inst in insts:
                si = inst.sync_info
                waits = list(si.on_wait) if si is not None and si.on_wait \
                    else []
                if len(waits) > limit:
                    keep = waits[-limit:]
                    extra = waits[:-limit]
                    for i in range(0, len(extra), limit):
                        chunk = extra[i:i + limit]
                        _WAITSPLIT_COUNTER[0] += 1
                        nop = mybir.InstNoOp(
                            name=f"waitsplit-{_WAITSPLIT_COUNTER[0]}",
                            ins=[], outs=[])
                        nop.engine = inst.engine
                        nop.sync_info = mybir.SyncInfo(
                            on_wait=chunk, on_update=[])
                        out.append(nop)
                    si.on_wait = keep
                    n_split += 1
                    changed = True
                out.append(inst)
            if changed:
                bb.instructions = out
    return n_split


_CACHE = {}


def _get_module(n_loc, split=True, debug=False):
    key = (n_loc, split, debug)
    if key not in _CACHE:
        nc = build_module(n_loc, debug=debug)
        if split:
            _split_waits(nc, limit=1)
        _CACHE[key] = nc
    return _CACHE[key]


def make_in_maps(inputs, n_loc=None):
    x = np.ascontiguousarray(np.asarray(inputs["x"], dtype=np.float32))
    qkv_w = np.asarray(inputs["qkv_w"], dtype=np.float32)
    qkv_b = np.asarray(inputs["qkv_b"], dtype=np.float32)
    out_w = np.asarray(inputs["out_w"], dtype=np.float32)
    out_b = np.asarray(inputs["out_b"], dtype=np.float32)
    gn_w = np.asarray(inputs["gn_weight"], dtype=np.float32)
    gn_b = np.asarray(inputs["gn_bias"], dtype=np.float32)

    n_tot = int(np.prod(x.shape[2:]))
    if n_loc is None:
        n_loc = n_tot // N_CORES
    xf = x.reshape(B, C, n_tot)

    wqk_t = np.ascontiguousarray(_round_tf32(qkv_w[0:2 * C].T))
    wv_t = np.ascontiguousarray(_round_tf32(qkv_w[2 * C:3 * C].T))
    wo_t = np.ascontiguousarray(_round_tf32(out_w.T))
    qkb = np.ascontiguousarray(qkv_b[0:2 * C].reshape(1, 2 * C))
    vb2 = np.ascontiguousarray(qkv_b[2 * C:3 * C].reshape(CO, P).T)
    ob2 = np.ascontiguousarray(out_b.reshape(CO, P).T)
    gnw2 = np.ascontiguousarray(gn_w.reshape(CO, P).T)
    gnb2 = np.ascontiguousarray(gn_b.reshape(CO, P).T)

    shared = dict(wqk_t=wqk_t, wv_t=wv_t, wo_t=wo_t, qkb=qkb, vb2=vb2,
                  ob2=ob2, gnw2=gnw2, gnb2=gnb2)
    in_maps = []
    for c in range(N_CORES):
        sl = np.ascontiguousarray(xf[:, :, c * n_loc:(c + 1) * n_loc])
        in_maps.append({"xin": sl, **shared})
    return in_maps


def run(inputs, n_loc=None, **kw):
    x = np.asarray(inputs["x"])
    n_tot = int(np.prod(x.shape[2:]))
    if n_loc is None:
        n_loc = n_tot // N_CORES
    nc = _get_module(n_loc)
    in_maps = make_in_maps(inputs, n_loc)
    res = bass_utils.run_bass_kernel_spmd(
        nc, in_maps, core_ids=list(range(N_CORES)), **kw)
    y = np.concatenate([res.results[c]["yout"] for c in range(N_CORES)],
                       axis=2)
    return y, res


def kernel(**inputs) -> np.ndarray:
    x = np.asarray(inputs["x"])
    y, _ = run(inputs)
    return y.reshape(x.shape).astype(np.asarray(x).dtype)

